# revision 11
# baseline (speedup 1.0000x reference)
"""Trainium2 Bass kernel for nn_AttentionBlock (GroupNorm + 1x1-conv QKV
self-attention + 1x1-conv out-proj + residual).

Full input shapes: x (8, 256, 64, 64) f32, gn_weight/gn_bias (256,),
qkv_w (768, 256), qkv_b (768,), out_w (256, 256), out_b (256,).

Sharding: data-parallel over batch - one batch item per NeuronCore (8 cores).

Design (v3, fp8 DoubleRow, folded out-proj):
  * Scores are computed as S^T[j,i] = xn_j . u_i with u = (Wk^T Wq) xn,
    folding the q and k convs into ONE conv against the host-precomputed
    256x256 matrix M = Wk^T Wq.  Per-query-constant bias terms drop out of
    softmax; the per-j term xn_j.(Wk^T qb) is produced as an extra output
    column of the v conv (zero when qkv_b == 0, the fast path).
  * The out-proj is folded into the v conv on the host: W2 = out_w @ Wv,
    so PV accumulation directly yields the unnormalized output channels
    and the separate out-proj matmul (and the attn fp8 requantize) vanish.
    yo = pvp * (1/den) + (x + obias), with the obias add done on ACT.
  * All large matmuls run in fp8e4 (e4m3) with MatmulPerfMode.DoubleRow:
    the two 128-channel K-tiles are packed per instruction (K=256 per
    matmul, N=512 columns at ~259ns measured).
  * exp runs on ACT in N=1024 batches with scale=1/16 and bias=-ln4.
  * The softmax denominator is a DoubleRow ones-matmul on the PE chained
    over j-pairs (lhsT = 1/32); rb = 1/den via dcopy(x32) + reciprocal.
  * GN rstd uses exp(-0.5*ln(var+eps)) so ACT only ever needs the
    {Ln,Exp,Copy} table set -> single ACT_TABLE_LOAD, none on the
    critical path.  GN small-op chain fused; xn writes split DVE/ACT.
  * x stays resident in SBUF (own pool) for the residual - no re-DMA.
  * A junk-matmul warmup chain paced by the x DMA chunks keeps the PE
    p-state ramped so the first scores matmuls run at full clock.
  * Last block's y DMAs issue from the Scalar engine's DGE (idle there)
    instead of the backlogged Sync sequencer.
"""

import math

import ml_dtypes
import numpy as np

import concourse.bass as bass
import concourse.tile as tile
from concourse import bacc, mybir
from concourse.bass_utils import run_bass_kernel_spmd

F32 = mybir.dt.float32
F32R = mybir.dt.float32r
FP8 = mybir.dt.float8e4
AF = mybir.ActivationFunctionType
OP = mybir.AluOpType
DR = mybir.MatmulPerfMode.DoubleRow
DRSI = mybir.MatmulPerfMode.DoubleRowSwInterleave
NPFP8 = ml_dtypes.float8_e4m3

B = 8          # batch (= cores)
C = 256        # channels
P = 128        # partitions
NCC = C // P   # channel chunks (2)
G = 32         # groups
GS = C // G    # channels per group (8)
GPC = P // GS  # groups per partition chunk (16)
EPS = 1e-5
VN = 272       # v conv cols: 256 ch + bias col + pad to 16B multiple
LN4 = float(np.log(16.0))   # exp downscale: keeps es under fp8e4 max (240)
ESCALE = 1.0 / 16.0   # attention scale 1/sqrt(C)


def build(hw=4096, iblk=512, has_qkv_bias=False):
    """Build the per-core Bass program. hw = pixels per image (4096 full)."""
    assert hw % 512 == 0 and hw % iblk == 0 and iblk == 512
    njt = hw // P        # j tiles of 128 (32 full size)
    npair = njt // 2     # j-tile pairs (16)
    nib = hw // iblk     # i blocks (8)
    nxc = hw // 512      # x chunks per cc

    nc = bacc.Bacc("TRN2", target_bir_lowering=False, debug=False, num_devices=B)

    x_d = nc.dram_tensor("x", [NCC, nxc, P, 512], F32, kind="ExternalInput").ap()
    mwt_d = nc.dram_tensor("mwt", [P, NCC, NCC, P], FP8, kind="ExternalInput").ap()
    wvt_d = nc.dram_tensor("wvt", [P, NCC, VN], FP8, kind="ExternalInput").ap()
    obias_d = nc.dram_tensor("obias", [P, NCC], F32, kind="ExternalInput").ap()
    gn_w_d = nc.dram_tensor("gn_w", [P, NCC], F32, kind="ExternalInput").ap()
    gn_b_d = nc.dram_tensor("gn_b", [P, NCC], F32, kind="ExternalInput").ap()
    gmask_d = nc.dram_tensor("gmask", [P, GPC], F32, kind="ExternalInput").ap()
    gmaskT_d = nc.dram_tensor("gmaskT", [GPC, P], F32, kind="ExternalInput").ap()
    y_d = nc.dram_tensor("y", [NCC, nxc, P, 512], F32, kind="ExternalOutput").ap()

    with tile.TileContext(nc) as tc:
        with (
            tc.tile_pool(name="const", bufs=1) as cst,
            tc.tile_pool(name="x", bufs=1) as xp,
            tc.tile_pool(name="u", bufs=1) as up,
            tc.tile_pool(name="v", bufs=1) as vp,
            tc.tile_pool(name="xn", bufs=1) as xnp,
            tc.tile_pool(name="es", bufs=2) as esp,
            tc.tile_pool(name="work", bufs=2) as wp,
            tc.tile_pool(name="stat", bufs=2) as sp,
            tc.tile_pool(name="ps_s", bufs=2, space="PSUM") as ps_s,
            tc.tile_pool(name="ps_pv", bufs=1, space="PSUM") as ps_pv,
            tc.tile_pool(name="ps_d", bufs=1, space="PSUM") as ps_d,
            tc.tile_pool(name="ps_m", bufs=1, space="PSUM") as ps_m,
        ):
            # ---- constants / weights to SBUF ----
            mwt = cst.tile([P, NCC, NCC, P], FP8)
            wvt = cst.tile([P, NCC, VN], FP8)
            obias = cst.tile([P, NCC], F32)
            gn_w = cst.tile([P, NCC], F32)
            gn_b = cst.tile([P, NCC], F32)
            gmask = cst.tile([P, GPC], F32)
            gmaskT = cst.tile([GPC, P], F32)
            dones = cst.tile([P, NCC, P], FP8)   # den lhsT = 1/32, M=128
            wones = cst.tile([P, P], F32)        # PE warmup lhsT (junk)
            eps_t = cst.tile([P, 1], F32)
            nln4 = cst.tile([P, 1], F32)         # exp bias: -ln4
            nc.vector.memset(dones, 1.0 / 32.0)
            nc.vector.memset(wones, 1.0)
            nc.vector.memset(eps_t, EPS)
            nc.vector.memset(nln4, -LN4)

            # big persistent tensors
            u_sb = up.tile([P, NCC, hw], FP8)      # u = M @ xn (scores rhs)
            v_sb = vp.tile([P, njt, VN], FP8)      # v' = (out_w Wv) xn, (j,o)
            xn = xnp.tile([P, NCC, hw], FP8)       # normalized x (fp8)
            xs = xp.tile([P, NCC, hw], F32)        # x resident (residual)

            # x chunks first (16 issues pace the DMA), weights after
            for cc in range(NCC):
                for h2 in range(nxc):
                    nc.sync.dma_start(
                        out=xs[:, cc, h2 * 512:(h2 + 1) * 512],
                        in_=x_d[cc, h2],
                    )
            nc.sync.dma_start(out=mwt, in_=mwt_d)
            nc.sync.dma_start(out=wvt, in_=wvt_d)
            nc.sync.dma_start(out=obias, in_=obias_d)
            nc.sync.dma_start(out=gn_w, in_=gn_w_d)
            nc.sync.dma_start(out=gn_b, in_=gn_b_d)
            nc.sync.dma_start(out=gmask, in_=gmask_d)
            nc.sync.dma_start(out=gmaskT, in_=gmaskT_d)

            # ---- PE warmup: junk f32 matmuls chained to the arriving x
            # chunks keep the PE p-state ramped until real work starts.
            # Results land in the (not yet used) den PSUM bank, never read.
            wps = ps_d.tile([P, 512], F32, tag="dbc", name="warm")
            nw = NCC * nxc
            for k in range(nw):
                cc, h2 = divmod(k, nxc)
                nc.tensor.matmul(
                    wps[:, 0:256],
                    wones,
                    xs[:, cc, h2 * 512:h2 * 512 + 256],
                    start=(k == 0), stop=(k == nw - 1),
                )

            # ---- GroupNorm: per-cc bn_stats chase the DMA; the two per-cc
            # reduction chains interleave stage-by-stage.  rstd comes from
            # exp(-0.5*ln(var+eps)) so ACT stays on the {Ln,Exp,Copy} table.
            ab_l = []
            for cc in range(NCC):
                stats = sp.tile([P, nxc, 6], F32, tag="bnst", name=f"st{cc}")
                for sg in range(nxc):
                    nc.vector.bn_stats(
                        out=stats[:, sg, :], in_=xs[:, cc, sg * 512:(sg + 1) * 512]
                    )
                mv = sp.tile([P, 2], F32, tag="mv", name=f"mv{cc}")
                nc.vector.bn_aggr(out=mv, in_=stats)
                # col1 <- mean^2 + var (in place, fused)
                nc.vector.tensor_scalar(
                    out=mv[:, 1:2], in0=mv[:, 0:1],
                    scalar1=mv[:, 0:1], scalar2=mv[:, 1:2],
                    op0=OP.mult, op1=OP.add,
                )
                gsum = ps_s.tile([GPC, 2], F32, tag="mm", name=f"gs{cc}")
                nc.tensor.matmul(gsum, gmask, mv, start=True, stop=True)
                gstat = sp.tile([GPC, 2], F32, tag="gstat", name=f"gt{cc}")
                nc.vector.tensor_scalar(
                    out=gstat, in0=gsum, scalar1=1.0 / GS, scalar2=None,
                    op0=OP.mult,
                )
                gvar = sp.tile([GPC, 1], F32, tag="gvar", name=f"gv{cc}")
                nc.vector.tensor_mul(gvar, gstat[:, 0:1], gstat[:, 0:1])
                nc.vector.tensor_sub(gvar, gstat[:, 1:2], gvar)
                # rstd = exp(-0.5 * ln(var + eps)); Ln/Exp share the table set
                nc.scalar.activation(gvar, gvar, AF.Ln, bias=eps_t[0:GPC, :])
                gmr = sp.tile([GPC, 2], F32, tag="gmr", name=f"gm{cc}")
                nc.scalar.activation(gmr[:, 1:2], gvar, AF.Exp, scale=-0.5)
                nc.vector.tensor_copy(gmr[:, 0:1], gstat[:, 0:1])
                bc = ps_s.tile([P, 2], F32, tag="mm", name=f"bc{cc}")
                nc.tensor.matmul(bc, gmaskT, gmr, start=True, stop=True)
                a_t = sp.tile([P, 1], F32, tag="a", name=f"a{cc}")
                b_t = sp.tile([P, 1], F32, tag="b", name=f"b{cc}")
                nc.vector.tensor_mul(a_t, bc[:, 1:2], gn_w[:, cc:cc + 1])
                nc.vector.tensor_mul(b_t, bc[:, 0:1], a_t)
                nc.vector.tensor_sub(b_t, gn_b[:, cc:cc + 1], b_t)
                ab_l.append((a_t, b_t))
                # first 512 xn cols of this cc right away (unblocks uconv
                # jb0 + scores pairs 0-1); cc0 on DVE, cc1 on ACT
                if cc == 0:
                    nc.vector.tensor_scalar(
                        out=xn[:, 0, 0:512], in0=xs[:, 0, 0:512],
                        scalar1=a_t, scalar2=b_t, op0=OP.mult, op1=OP.add,
                    )
                else:
                    nc.scalar.activation(
                        xn[:, 1, 0:512], xs[:, 1, 0:512], AF.Identity,
                        bias=b_t, scale=a_t,
                    )
            a0, b0 = ab_l[0]
            a1, b1 = ab_l[1]
            nc.vector.tensor_scalar(
                out=xn[:, 0, 512:hw], in0=xs[:, 0, 512:hw],
                scalar1=a0, scalar2=b0, op0=OP.mult, op1=OP.add,
            )
            nc.scalar.activation(
                xn[:, 1, 512:hw], xs[:, 1, 512:hw], AF.Identity, bias=b1, scale=a1,
            )

            # ---- u conv: u = (1/16) * mwt^T @ xn, DoubleRow over the two
            # 128-channel K-tiles.  Only jb=0 runs before block 0 (scores of
            # block ib need just u chunk jb=ib); jb 1-7 overlap block 0. ----
            def emit_uconv(jb):
                pu = ps_s.tile([P, NCC, 512], F32, tag="mm", name=f"pu{jb}")
                for oc in range(NCC):
                    nc.tensor.matmul(
                        pu[:, oc, :],
                        mwt[:, :, oc, :],
                        xn[:, :, jb * 512:(jb + 1) * 512],
                        start=True, stop=True, perf_mode=DR,
                    )
                nc.vector.tensor_scalar(
                    out=u_sb[:, :, jb * 512:(jb + 1) * 512],
                    in0=pu, scalar1=1.0 / 16.0, scalar2=None, op0=OP.mult,
                )

            st = {}

            def emit_scores_pair(ib, g, s_ps):
                isl = slice(ib * iblk, (ib + 1) * iblk)
                for q in range(2):
                    jt = 2 * g + q
                    nc.tensor.matmul(
                        s_ps[:, q, :],
                        xn[:, :, jt * P:(jt + 1) * P],
                        u_sb[:, :, isl],
                        start=True, stop=True, perf_mode=DR,
                    )

            def emit_exp_pair(ib, g, s_ps):
                es = st[ib]["es"]
                if not has_qkv_bias:
                    nc.scalar.activation(
                        es[:, 2 * g:2 * g + 2, :], s_ps, AF.Exp,
                        scale=ESCALE, bias=nln4,
                    )
                else:
                    bcol = st["bcol"]
                    for q in range(2):
                        jt = 2 * g + q
                        nc.scalar.activation(
                            es[:, jt, :], s_ps[:, q, :], AF.Exp,
                            scale=ESCALE, bias=bcol[:, jt:jt + 1],
                        )

            def emit_den_pair(ib, g):
                # denominator, broadcast to all 128 rows by an all-1/32 lhsT
                es = st[ib]["es"]
                nc.tensor.matmul(
                    st[ib]["dbc"],
                    dones,
                    es[:, 2 * g:2 * g + 2, :],
                    start=(g == 0), stop=(g == npair - 1), perf_mode=DRSI,
                )

            def emit_pv_pair(ib, g):
                es = st[ib]["es"]
                pvp = st[ib]["pvp"]
                for oc in range(NCC):
                    nc.tensor.matmul(
                        pvp[:, oc, :],
                        v_sb[:, 2 * g:2 * g + 2, oc * P:(oc + 1) * P],
                        es[:, 2 * g:2 * g + 2, :],
                        start=(g == 0), stop=(g == npair - 1), perf_mode=DR,
                    )

            # ---- per-block tail ops (for block ib, run during block ib+1).
            # rb = 1/den via dcopy(x32)+reciprocal; yo = pvp*rb + (x+obias).
            def emit_dcopy(ib):
                dsb = wp.tile([P, iblk], F32, tag="dsb", name=f"dsb{ib}")
                nc.vector.tensor_scalar(
                    out=dsb, in0=st[ib]["dbc"], scalar1=32.0, scalar2=None,
                    op0=OP.mult,
                )
                st[ib]["dsb"] = dsb

            def emit_recip(ib):
                rb = wp.tile([P, iblk], F32, tag="rb", name=f"rb{ib}")
                nc.vector.reciprocal_approx_fast(rb, st[ib]["dsb"])
                st[ib]["rb"] = rb

            def emit_xob(ib, o2):
                # residual + out-bias staged on ACT (x is SBUF-resident)
                xob = wp.tile([P, iblk], F32, tag=f"xob{o2}", name=f"xo{ib}_{o2}")
                nc.vector.tensor_scalar(
                    out=xob, in0=xs[:, o2, ib * iblk:(ib + 1) * iblk],
                    scalar1=obias[:, o2:o2 + 1], scalar2=None, op0=OP.add,
                )
                st[ib][f"xob{o2}"] = xob

            def emit_yv(ib, o2):
                yv = wp.tile([P, iblk], F32, tag=f"yv{o2}", name=f"yv{ib}_{o2}")
                nc.vector.tensor_mul(yv, st[ib]["pvp"][:, o2, :], st[ib]["rb"])
                st[ib][f"yv{o2}"] = yv

            def emit_yo(ib, o2):
                yo = wp.tile([P, iblk], F32, tag=f"yo{o2}", name=f"yo{ib}_{o2}")
                if ib == nib - 1:
                    # last block: halves, each DMA'd (scalar DGE) as soon as
                    # its add lands, so the writes overlap the tail chain
                    for hh in range(2):
                        hs = slice(hh * 256, hh * 256 + 256)
                        nc.vector.tensor_add(
                            yo[:, hs], st[ib][f"yv{o2}"][:, hs],
                            st[ib][f"xob{o2}"][:, hs],
                        )
                        nc.scalar.dma_start(
                            out=y_d[o2, ib][:, hs], in_=yo[:, hs]
                        )
                else:
                    nc.vector.tensor_add(
                        yo, st[ib][f"yv{o2}"], st[ib][f"xob{o2}"]
                    )
                    nc.sync.dma_start(out=y_d[o2, ib], in_=yo)

            def start_block(ib):
                st[ib] = {
                    "es": esp.tile([P, njt, iblk], FP8, tag="es", name=f"es{ib}"),
                    "pvp": ps_pv.tile([P, NCC, iblk], F32, tag="pv", name=f"pv{ib}"),
                    "dbc": ps_d.tile([P, iblk], F32, tag="dbc", name=f"db{ib}"),
                }

            # v conv: (j, o) layout with W2 = out_w @ Wv folded in; psum
            # padded to bank width; both j-tiles written back in one DVE op
            def emit_vconv_pair(g, pool):
                pv = pool.tile([P, NCC, 512], F32, tag="mm", name=f"vc{g}")
                for q in range(2):
                    jt = 2 * g + q
                    nc.tensor.matmul(
                        pv[:, q, 0:VN],
                        xn[:, :, jt * P:(jt + 1) * P],
                        wvt,
                        start=True, stop=True, perf_mode=DR,
                    )
                nc.vector.tensor_scalar(
                    out=v_sb[:, 2 * g:2 * g + 2, :], in0=pv[:, :, 0:VN],
                    scalar1=1.0 / 16.0, scalar2=None, op0=OP.mult,
                )

            if has_qkv_bias:
                # b_j = xn_j . (Wk^T qb) comes out as v_sb col 256
                bcol = cst.tile([P, njt], F32)
                st["bcol"] = bcol

            emit_uconv(0)
            if has_qkv_bias:
                # bias path reads v_sb col 256 for all j-tiles before block 0,
                # so the full v conv must run up front here
                for g in range(npair):
                    emit_vconv_pair(g, ps_s)
                nc.vector.tensor_copy(st["bcol"], v_sb[:, :, 256])
                nc.vector.tensor_scalar(
                    out=st["bcol"], in0=st["bcol"],
                    scalar1=16.0, scalar2=-LN4, op0=OP.mult, op1=OP.add,
                )

            # ---- main pipeline: global slot schedule.
            # Work of block ib at block-relative emission slots:
            #   scores/exp pair g: slot g      den round k: slot 2k+3
            #   pv pairs 0-10: slots 5-15; 11,12,13: +16; 14,15: +17
            #   copyr: +18  combine: +19  recip/yv0/yv1: +20  yo: +21/+22
            # pvp's last read (yv1) is emitted at +20, before the next
            # block's first pv write at +21; dbc's last read (copyr, +18)
            # precedes the next block's den round 0 write at +19. ----
            nslots = nib * npair + 8
            for s in range(nslots):
                blk, r = divmod(s, npair)
                if blk < nib:
                    if r == 0:
                        start_block(blk)
                    s_ps = ps_s.tile(
                        [P, 2, iblk], F32, tag="mm", name=f"s{blk}_{r}"
                    )
                    emit_scores_pair(blk, r, s_ps)
                    emit_exp_pair(blk, r, s_ps)
                    if blk == 0 and r <= 7 and not has_qkv_bias:
                        emit_vconv_pair(r, ps_s)
                    if blk == 0 and r == 8:
                        emit_uconv(1)
                    if 1 <= blk <= nib - 2 and r == 6:
                        emit_uconv(blk + 1)
                    if blk == 0 and not has_qkv_bias:
                        jt = njt // 2 + r
                        pvq = ps_m.tile([P, 512], F32, tag="mm", name=f"vq{jt}")
                        nc.tensor.matmul(
                            pvq[:, 0:VN],
                            xn[:, :, jt * P:(jt + 1) * P],
                            wvt,
                            start=True, stop=True, perf_mode=DR,
                        )
                        nc.vector.tensor_scalar(
                            out=v_sb[:, jt, :], in0=pvq[:, 0:VN],
                            scalar1=1.0 / 16.0, scalar2=None, op0=OP.mult,
                        )
                d = s - 2
                if 0 <= d < nib * npair:
                    emit_den_pair(d // npair, d % npair)
                # pv: current block pairs 0-10 at slots 5-15, prev block
                # pairs 11-15 at slots 0,0,0,1,1
                if blk < nib and 5 <= r <= 15:
                    emit_pv_pair(blk, r - 5)
                pb = blk - 1
                if 0 <= pb < nib:
                    if r == 0:
                        emit_pv_pair(pb, 11)
                        emit_pv_pair(pb, 12)
                        emit_pv_pair(pb, 13)
                        emit_xob(pb, 0)
                    elif r == 1:
                        emit_pv_pair(pb, 14)
                        emit_pv_pair(pb, 15)
                        emit_dcopy(pb)
                        emit_xob(pb, 1)
                    elif r == 2:
                        emit_recip(pb)
                        emit_yv(pb, 0)
                    elif r == 3:
                        emit_yv(pb, 1)
                    elif r == 4:
                        emit_yo(pb, 0)
                    elif r == 5:
                        emit_yo(pb, 1)
                        if pb - 1 in st:
                            del st[pb - 1]

    nc.compile()
    return nc


def prep_inputs(x, gn_weight, gn_bias, qkv_w, qkv_b, out_w, out_b, hw=4096):
    """Host-side layout prep. Returns per-core input maps."""
    b = x.shape[0]
    x = np.asarray(x, np.float32)
    qkv_w = np.asarray(qkv_w, np.float32)
    qkv_b = np.asarray(qkv_b, np.float32)
    out_w = np.asarray(out_w, np.float32)
    out_b = np.asarray(out_b, np.float32)
    wq = qkv_w[:C]
    wk = qkv_w[C:2 * C]
    wv = qkv_w[2 * C:]
    qb = qkv_b[:C]
    vb = qkv_b[2 * C:]

    m_mat = (wk.T @ wq).astype(np.float32)          # S^T = xn^T (M xn)
    m16 = 16.0 * m_mat
    # mwt[k, q, oc, m] = m16[oc*128+m, q*128+k]
    mwt = np.ascontiguousarray(
        m16.reshape(NCC, P, NCC, P).transpose(3, 2, 0, 1)
    ).astype(NPFP8)
    # wv_ext rows: 16*(out_w @ Wv) (256), 16*(Wk^T qb) bias col, zero pad
    wv_ext = np.zeros((VN, C), np.float32)
    wv_ext[:C] = 16.0 * (out_w @ wv)
    wv_ext[C] = 16.0 * (wk.T @ qb)
    # wvt[k, q, n] = wv_ext[n, q*128+k]
    wvt = np.ascontiguousarray(
        wv_ext.reshape(VN, NCC, P).transpose(2, 1, 0)
    ).astype(NPFP8)
    ob = out_b + out_w @ vb
    obias = np.ascontiguousarray(ob.reshape(NCC, P).T).astype(np.float32)
    gn_w2 = np.ascontiguousarray(gn_weight.reshape(NCC, P).T).astype(np.float32)
    gn_b2 = np.ascontiguousarray(gn_bias.reshape(NCC, P).T).astype(np.float32)
    gmask = np.zeros((P, GPC), np.float32)
    gmask[np.arange(P), np.arange(P) // GS] = 1.0
    gmaskT = np.ascontiguousarray(gmask.T)

    shared = dict(
        mwt=mwt, wvt=wvt, obias=obias,
        gn_w=gn_w2, gn_b=gn_b2, gmask=gmask, gmaskT=gmaskT,
    )
    in_maps = []
    for i in range(b):
        m = dict(shared)
        m["x"] = np.ascontiguousarray(
            x[i].reshape(C, hw).reshape(NCC, P, hw // 512, 512)
            .transpose(0, 2, 1, 3)
        ).astype(np.float32)
        in_maps.append(m)
    return in_maps


_NC_CACHE = {}


def get_nc(hw=4096, iblk=512, has_qkv_bias=False):
    key = (hw, iblk, has_qkv_bias)
    if key not in _NC_CACHE:
        _NC_CACHE[key] = build(hw, iblk, has_qkv_bias)
    return _NC_CACHE[key]


def kernel(x, gn_weight, gn_bias, qkv_w, qkv_b, out_w, out_b):
    b, c, h, w = x.shape
    assert (b, c) == (B, C)
    hw = h * w
    has_qkv_bias = bool(np.any(np.asarray(qkv_b) != 0))
    nc = get_nc(hw=hw, has_qkv_bias=has_qkv_bias)
    in_maps = prep_inputs(x, gn_weight, gn_bias, qkv_w, qkv_b, out_w, out_b, hw=hw)
    res = run_bass_kernel_spmd(nc, in_maps, core_ids=list(range(B)))
    out = np.stack(
        [
            np.asarray(res.results[i]["y"])
            .reshape(NCC, hw // 512, P, 512)
            .transpose(0, 2, 1, 3)
            .reshape(C, h, w)
            for i in range(b)
        ]
    ).astype(np.float32)
    return out
